# revision 43
# baseline (speedup 1.0000x reference)
"""Trainium2 Bass kernel for nn_Attention_32091995636193.

Dense transformer attention block:
  qkv = x @ qkv_w.T ; per-head LN(q), LN(k) over head_dim ; k centered over
  seq ; softmax(q*scale @ k^T) @ v ; out @ proj_w.T + proj_b.

Sharding over 8 NeuronCores: data parallel on batch (B=2) x tensor parallel
on heads (16 heads -> 4 per core). Core c handles batch c//4, heads
4*(c%4) .. 4*(c%4)+3. Each core computes its partial projection output
[N, C]; the host sums the 4 partials per batch and adds proj_b.

v2 design (vs the 312us baseline):
  - exp split across ACT (native Exp, 9/16 tiles) and DVE (Schraudolph
    bit-trick: int16(logit*rs*184.665 + 16250.5) bitcast to bf16, 7/16)
    so softmax never limits the PE.
  - attention emitted in batches of 3 positions (6 same-shape scores then
    6 same-shape attnv matmuls): alternating PE geometries every position
    costs ~55ns/matmul in ldweights overlap, batching amortizes it. The
    attnv lag also covers the single-buffered U evacuation (PSUM: ps x3
    + U x1 = 8 banks).
  - qkv: wk|wq fused into one 512-wide matmul per k-step; LN stats via
    bn_stats on DVE; LN applies split DVE/ACT; stats->apply->transpose
    ladder pipelined against the PE in 4-tile groups.
  - softmax normalization: reciprocal on 128 partitions via DMA reshape
    (half-0 recip hides mid-attention), fp32r ones-matmul broadcast, DVE
    multiplies; projection tiles stream through fp16 SBUF to DRAM on the
    idle sync/gpsimd queues.
"""

import os
import sys

for _p in ("/opt/trn_rl_repo",):
    if _p not in sys.path and os.path.isdir(_p):
        sys.path.append(_p)

import numpy as np

# ---------------------------------------------------------------------------
# BIR legalizer: the pinned walrus build supports at most ONE sync wait per
# instruction, but Tile's scheduler attaches several. Split extra waits onto
# NoOp instructions inserted immediately before (same engine => same NX
# order => identical semantics).
# ---------------------------------------------------------------------------
import orjson


def _legalize_bir_json_bytes(raw: bytes) -> bytes:
    j = orjson.loads(raw)
    counter = 0
    for f in j.get("functions", []):
        for blk in f.get("blocks", []):
            insts = blk.get("instructions")
            if not insts:
                continue
            out = []
            for ins in insts:
                si = ins.get("sync_info")
                waits = si.get("on_wait") if si else None
                if waits and len(waits) > 1:
                    engine = ins.get("engine")
                    for w in waits[:-1]:
                        counter += 1
                        nop = {
                            "name": f"WSPLIT-{counter}",
                            "opcode": "NoOp",
                            "engine": engine,
                            "ins": [],
                            "outs": [],
                            "sync_info": {"on_wait": [w], "on_update": []},
                        }
                        if "debug" in ins:
                            nop["debug"] = ins["debug"]
                        out.append(nop)
                    si["on_wait"] = [waits[-1]]
                out.append(ins)
            blk["instructions"] = out
    return orjson.dumps(j)


_PATCHED = False


def _install_patch():
    global _PATCHED
    if _PATCHED:
        return
    import concourse.bass as bass

    orig = bass.Bass.to_json_bytes

    def patched(self):
        return _legalize_bir_json_bytes(orig(self))

    bass.Bass.to_json_bytes = patched
    _PATCHED = True


# ---------------------------------------------------------------------------
# Problem constants (hardcoded per the harness contract)
# ---------------------------------------------------------------------------
B = 2
N = 2048
C = 1024
H = 16
D = 64
SCALE = D ** -0.5
EPS = 1e-5
NCORES = 8
HPC = H // 4          # heads per core = 4
DPC = HPC * D         # channels per core = 256
NT = N // 128         # 16 n-tiles
KT = C // 128         # 8 contraction tiles

# Schraudolph exp-as-bits constants (bf16 target: 2^7 mantissa scale)
A_BITS = 128.0 * 1.4426950408889634   # 128*log2(e)
B_BITS = 16250.5                      # 127*128 - 5.5 (minimax shift)

_nc_cache = {}


def _build_program(ln_trivial: bool):
    import concourse.bass as bass
    import concourse.mybir as mybir
    import concourse.tile as tile

    f32 = mybir.dt.float32
    fr = mybir.dt.float32r
    bf = mybir.dt.bfloat16
    i16 = mybir.dt.int16
    AX = mybir.AxisListType
    OP = mybir.AluOpType
    ACTF = mybir.ActivationFunctionType

    nc = bass.Bass()
    xt = nc.declare_dram_parameter("xt", [C, N], bf, isOutput=False)
    wq = nc.declare_dram_parameter("wq", [C, DPC], bf, isOutput=False)
    wk = nc.declare_dram_parameter("wk", [C, DPC], bf, isOutput=False)
    wv = nc.declare_dram_parameter("wv", [C, DPC], bf, isOutput=False)
    wp = nc.declare_dram_parameter("wp", [DPC, C], bf, isOutput=False)
    ident_in = nc.declare_dram_parameter("ident", [128, 128], bf, isOutput=False)
    ones64 = nc.declare_dram_parameter("ones64", [1, D], fr, isOutput=False)
    if not ln_trivial:
        gqb = nc.declare_dram_parameter("gqb", [128, 4, D], f32, isOutput=False)
        bqb = nc.declare_dram_parameter("bqb", [128, 4, D], f32, isOutput=False)
        gkb = nc.declare_dram_parameter("gkb", [128, 4, D], f32, isOutput=False)
        bkb = nc.declare_dram_parameter("bkb", [128, 4, D], f32, isOutput=False)
    f16 = mybir.dt.float16
    out = nc.declare_dram_parameter("out", [N, C], f16, isOutput=True)

    with tile.TileContext(nc) as tc:
        with tc.tile_pool(name="const", bufs=1) as cpool, \
             tc.tile_pool(name="persist", bufs=1) as bpool:

            ident = cpool.tile([128, 128], bf)
            nc.scalar.dma_start(ident[:], ident_in[:])

            ones_t = cpool.tile([1, D], fr)
            nc.scalar.dma_start(ones_t[:], ones64[:])
            eps_t = cpool.tile([128, 1], f32)
            nc.vector.memset(eps_t[:], EPS)
            eps64_t = cpool.tile([128, 1], f32)
            nc.vector.memset(eps64_t[:], D * EPS)

            # ---- persistent tensors (live into attention/proj) --------
            wp_s = bpool.tile([128, 2, C], bf, name="wp_s")
            v5 = bpool.tile([128, NT * HPC, 65], bf, name="v5")
            nc.vector.memset(v5[:, :, 64:65], 1.0)
            qT = [bpool.tile([128, N], bf, name=f"qT{p}") for p in range(2)]
            kT = [bpool.tile([128, N], bf, name=f"kT{p}") for p in range(2)]
            outT = [bpool.tile([128, N], bf, name=f"outT{p}") for p in range(2)]
            # softmax denominators + reciprocal broadcast buffers
            den_all = bpool.tile([1, 2, HPC, 1024], f32, name="den_all")
            denr = bpool.tile([1, 2, HPC, 1024], fr, name="denr")

            with tc.tile_pool(name="ph13", bufs=1) as wpool, \
                 tc.tile_pool(name="qkv_ps", bufs=3, space="PSUM") as qps, \
                 tc.tile_pool(name="stat_tmp", bufs=2) as stp, \
                 tc.tile_pool(name="tp_ps", bufs=2, space="PSUM") as tps:
                # ---- load inputs/weights for phases 1-3 ---------------
                # sync+gpsimd queues are idle at start: put the big xt
                # stream there so scalar/vector stay free for compute.
                # wk and wq land in adjacent halves of one [.., 512] tile so
                # each qkv k-step is 2 matmuls (one 512-wide) instead of 3
                # (fewer ldweights on the PE).
                wkq_s = wpool.tile([128, KT, 2, DPC], bf, name="wkq_s")
                wv_s = wpool.tile([128, KT, DPC], bf, name="wv_s")
                xt_s = wpool.tile([128, KT, N], bf, name="xt_s")
                xt_r = xt.rearrange("(k p) n -> p k n", p=128)
                for kc in range(KT):
                    eng = nc.sync if kc % 2 == 0 else nc.gpsimd
                    eng.dma_start(xt_s[:, kc:kc + 1], xt_r[:, kc:kc + 1])
                nc.sync.dma_start(wkq_s[:, :, 0], wk.rearrange("(k p) d -> p k d", p=128))
                nc.gpsimd.dma_start(wkq_s[:, :, 1], wq.rearrange("(k p) d -> p k d", p=128))
                nc.sync.dma_start(wv_s[:], wv.rearrange("(k p) d -> p k d", p=128))
                nc.scalar.dma_start(wp_s[:], wp.rearrange("(k p) n -> p k n", p=128))
                # warm the PE clock right before qkv: gate on the xt tile so
                # the burst doesn't fire early and decay while DMAs stream
                wut = qps.tile([128, 2, DPC], f32, tag="pkq")
                for i in range(45):
                    nc.tensor.matmul(wut[:], xt_s[:, 0, 0:128],
                                     xt_s[:, 0, 0:512], start=True, stop=True)
                if not ln_trivial:
                    gq_s = wpool.tile([128, 4, D], f32, name="gq_s")
                    nc.sync.dma_start(gq_s[:], gqb[:])
                    bq_s = wpool.tile([128, 4, D], f32, name="bq_s")
                    nc.sync.dma_start(bq_s[:], bqb[:])
                    gk_s = wpool.tile([128, 4, D], f32, name="gk_s")
                    nc.sync.dma_start(gk_s[:], gkb[:])
                    bk_s = wpool.tile([128, 4, D], f32, name="bk_s")
                    nc.sync.dma_start(bk_s[:], bkb[:])

                # ---- phases 1-3, half-batched ------------------------
                q_nat = wpool.tile([128, NT, 4, D], bf, name="q_nat")
                k_nat = wpool.tile([128, NT, 4, D], bf, name="k_nat")

                bnq = wpool.tile([128, NT, 4, 6], f32, name="bnq")
                bnk = wpool.tile([128, NT, 4, 6], f32, name="bnk")
                mu_q = bpool.tile([128, NT, 4], f32, name="mu_q")
                rs_q = bpool.tile([128, NT, 4], f32, name="rs_q")
                mu_k = bpool.tile([128, NT, 4], f32, name="mu_k")
                rs_k = bpool.tile([128, NT, 4], f32, name="rs_k")
                rs8 = bpool.tile([128, NT, 4], f32, name="rs8")
                mnr_q = bpool.tile([128, NT, 4], f32, name="mnr_q")

                def transpose_block(nat, dstT, s, tq, evac_eng):
                    """4 PE transposes covering n-cols [tq*512,(tq+1)*512) of
                    head pair s, plus one evac copy PSUM->SBUF."""
                    ptp = tps.tile([128, 4, 128], bf, tag="ptp")
                    for i in range(4):
                        t = 4 * tq + i
                        nc.tensor.transpose(
                            ptp[:, i], nat[:, t, 2 * s:2 * s + 2, :], ident[:])
                    if evac_eng == 0:
                        nc.vector.tensor_copy(
                            dstT[s][:, tq * 512:(tq + 1) * 512], ptp[:])
                    else:
                        nc.scalar.copy(
                            dstT[s][:, tq * 512:(tq + 1) * 512], ptp[:])

                def finalize_stats(bn, mu, rs, h8, kfold):
                    # bn 6-tuple = (n_e, m_e, n*var_e, n_o, m_o, n*var_o)
                    # mu = (m_e+m_o)/2
                    # E[x2] = (cv_e+cv_o)/64 + (m_e^2+m_o^2)/2 ; var = E[x2]-mu^2
                    # kfold: rs holds SCALE*rstd (exp scale; k needs no apply)
                    nt = h8.stop - h8.start
                    m_e = bn[:, h8, :, 1]
                    m_o = bn[:, h8, :, 4]
                    cv_e = bn[:, h8, :, 2]
                    cv_o = bn[:, h8, :, 5]
                    u = stp.tile([128, 8, 4], f32, tag="u", name="u")[:, 0:nt]
                    u2 = stp.tile([128, 8, 4], f32, tag="u2", name="u2")[:, 0:nt]
                    u3 = stp.tile([128, 8, 4], f32, tag="u3", name="u3")[:, 0:nt]
                    nc.vector.tensor_tensor(u[:], m_e, m_o, OP.add)
                    nc.vector.tensor_scalar(mu[:, h8], u[:], 0.5, None, OP.mult)
                    nc.vector.tensor_tensor(u[:], m_e, m_e, OP.mult)
                    nc.vector.scalar_tensor_tensor(u2[:], m_o, 1.0, m_o,
                                                   OP.mult, OP.mult)
                    nc.vector.tensor_tensor(u[:], u[:], u2[:], OP.add)
                    nc.vector.tensor_tensor(u2[:], cv_e, cv_o, OP.add)
                    # ex2 = u2/64 + u/2
                    nc.vector.tensor_scalar(u[:], u[:], 0.5, None, OP.mult)
                    nc.vector.scalar_tensor_tensor(u3[:], u2[:], 1.0 / D, u[:],
                                                   OP.mult, OP.add)
                    nc.vector.tensor_tensor(u[:], mu[:, h8], mu[:, h8], OP.mult)
                    nc.vector.tensor_tensor(u3[:], u3[:], u[:], OP.subtract)
                    if kfold:
                        # rs = 1/sqrt(64*var + 64*eps) = SCALE/sqrt(var+eps)
                        nc.scalar.activation(u[:], u3[:], ACTF.Sqrt,
                                             bias=eps64_t[:], scale=float(D))
                    else:
                        nc.scalar.activation(u[:], u3[:], ACTF.Sqrt,
                                             bias=eps_t[:], scale=1.0)
                    nc.vector.reciprocal(rs[:, h8], u[:])
                    if rs is rs_q:
                        # bias for the ACT-side LN applies: -mu*rstd
                        nc.vector.scalar_tensor_tensor(
                            mnr_q[:, h8], mu[:, h8], -1.0, rs[:, h8],
                            OP.mult, OP.mult)

                def apply_q(t, gs, eng=None):
                    for g in gs:
                        if g < 2:
                            nc.vector.tensor_scalar(
                                q_nat[:, t, g], q_nat[:, t, g],
                                mu_q[:, t, g:g + 1], rs_q[:, t, g:g + 1],
                                OP.subtract, OP.mult)
                        else:
                            nc.scalar.activation(
                                q_nat[:, t, g], q_nat[:, t, g], ACTF.Identity,
                                bias=mnr_q[:, t, g:g + 1],
                                scale=rs_q[:, t, g:g + 1])
                    if not ln_trivial:
                        nc.vector.tensor_mul(q_nat[:, t], q_nat[:, t], gq_s[:])
                        nc.vector.tensor_add(q_nat[:, t], q_nat[:, t], bq_s[:])

                def qkv_tile(t):
                    ts_ = slice(t * 128, (t + 1) * 128)
                    pkq = qps.tile([128, 2, DPC], f32, tag="pkq")
                    pv = qps.tile([128, DPC], f32, tag="pv")
                    for kc in range(KT):
                        nc.tensor.matmul(pkq[:], xt_s[:, kc, ts_],
                                         wkq_s[:, kc].rearrange("p a b -> p (a b)"),
                                         start=kc == 0, stop=kc == KT - 1)
                    for kc in range(KT):
                        nc.tensor.matmul(pv[:], xt_s[:, kc, ts_], wv_s[:, kc, :],
                                         start=kc == 0, stop=kc == KT - 1)
                    # PSUM evac on ACT; LN stats via one bn_stats per
                    # tensor (count/mean/var of even+odd element halves)
                    for (pp, bn, natd) in ((pkq[:, 0], bnk, k_nat),
                                           (pkq[:, 1], bnq, q_nat)):
                        pg = pp.rearrange("p (g d) -> p g d", g=4)
                        nc.scalar.copy(natd[:, t], pg)
                        for g in range(4):
                            nc.vector.bn_stats(bn[:, t, g], pg[:, g])
                    nc.scalar.copy(v5[:, t * HPC:(t + 1) * HPC, 0:64],
                                   pv[:].rearrange("p (g d) -> p g d", g=4))

                def apply_k(trange):
                    if ln_trivial:
                        return
                    for t in trange:
                        for g in range(4):
                            nc.vector.tensor_scalar(
                                k_nat[:, t, g], k_nat[:, t, g],
                                mu_k[:, t, g:g + 1], rs_k[:, t, g:g + 1],
                                OP.subtract, OP.mult)
                        nc.vector.tensor_mul(k_nat[:, t], k_nat[:, t], gk_s[:])
                        nc.vector.tensor_add(k_nat[:, t], k_nat[:, t], bk_s[:])

                # Pipelined phases 1-3: stats finalize + LN applies of each
                # t-group run on the DVE while the PE streams the next qkv
                # group / the transpose ladder, so the PE never waits.
                for t in range(0, 8):
                    qkv_tile(t)
                finalize_stats(bnq, mu_q, rs_q, slice(0, 8), False)
                for t in range(0, 8):
                    apply_q(t, range(4))
                finalize_stats(bnk, mu_k, rs_k, slice(0, 8), ln_trivial)
                nc.vector.tensor_scalar(rs8[:, 0:8], rs_k[:, 0:8],
                                        A_BITS, None, OP.mult)
                apply_k(range(0, 8))
                for t in range(8, 12):
                    qkv_tile(t)
                # half-0 k transposes: ACT copies for t0-7 are long done
                for s in range(2):
                    for tq in (0, 1):
                        transpose_block(k_nat, kT, s, tq, tq % 2)
                finalize_stats(bnq, mu_q, rs_q, slice(8, 12), False)
                for t in range(8, 12):
                    apply_q(t, range(4))
                for t in range(12, 16):
                    qkv_tile(t)
                finalize_stats(bnq, mu_q, rs_q, slice(12, 16), False)
                for t in range(12, 16):
                    apply_q(t, range(4))
                finalize_stats(bnk, mu_k, rs_k, slice(8, 16), ln_trivial)
                nc.vector.tensor_scalar(rs8[:, 8:16], rs_k[:, 8:16],
                                        A_BITS, None, OP.mult)
                apply_k(range(8, 16))
                # transpose ladder: k half-1, then only the q blocks the
                # first two attention chunks need (s0 tq0,1); the remaining
                # six q blocks are injected into early attention batches
                # (ps-tag PSUM slots), dissolving the post-qkv stall.
                for s in range(2):
                    for tq in (2, 3):
                        transpose_block(k_nat, kT, s, tq, tq % 2)
                for tq in (0, 1):
                    transpose_block(q_nat, qT, 0, tq, tq % 2)

                if not ln_trivial:
                    # center k over sequence (softmax-invariant, kept only
                    # for the general gamma/beta path)
                    with tc.tile_pool(name="ctr", bufs=1) as ctr:
                        for p in range(2):
                            rsum = ctr.tile([128, 1], f32, tag="rsum")
                            nc.vector.tensor_reduce(rsum[:], kT[p][:], AX.X, OP.add)
                            mean = ctr.tile([128, 1], f32, tag="mean")
                            nc.vector.tensor_scalar(mean[:], rsum[:], 1.0 / N,
                                                    None, OP.mult)
                            nc.vector.tensor_scalar(kT[p][:], kT[p][:], mean[:],
                                                    None, OP.subtract)

            # ---- attention, then normalize/proj/out ------------------
            with tc.tile_pool(name="den_pool", bufs=2) as dpool:

                def emit_recip(nh):
                    # spread the 4096 single-partition denominators over 128
                    # partitions via DMA, reciprocal, DMA back (as fp32r for
                    # the ones-matmul broadcast).
                    den128 = dpool.tile([128, 32], f32, tag="den128")
                    nc.sync.dma_start(den128[:],
                                      den_all[:, nh].rearrange("o h f -> o (h f)"))
                    der128 = dpool.tile([128, 32], fr, tag="der128")
                    with nc.allow_low_precision("softmax recip"):
                        nc.vector.reciprocal(der128[:], den128[:])
                    nc.sync.dma_start(
                        denr[:, nh].rearrange("o h f -> o (h f)"), der128[:])

                with tc.tile_pool(name="exp_pool", bufs=4) as epool, \
                     tc.tile_pool(name="att_ps", bufs=1, space="PSUM") as aps:
                    # nh-major: all heads' first 1024 cols finish halfway in
                    chunks = [(h, nh) for nh in range(2) for h in range(HPC)]
                    seq = [(ci, mt) for ci in range(len(chunks))
                           for mt in range(NT)]
                    DVE_MT = frozenset((1, 3, 5, 7, 9, 11, 13))
                    Us = {}
                    exs = {}
                    wps = aps.tile([128, 2, 512], f32, tag="ps", bufs=3,
                                   name="wps")
                    for i in range(10):
                        nc.tensor.matmul(wps[:, i % 2], kT[0][:, 0:128],
                                         kT[0][:, 0:512], start=True, stop=True)

                    def emit_attnv(ci, mt):
                        h, nh = chunks[ci]
                        exv = exs.pop((ci, mt))
                        for j in range(2):
                            nc.tensor.matmul(Us[ci][:, j * 512:(j + 1) * 512],
                                             v5[:, mt * HPC + h, :],
                                             exv[:, j * 512:(j + 1) * 512],
                                             start=(mt == 0), stop=(mt == NT - 1))
                        if mt == NT - 1:
                            p = h // 2
                            off = 64 * (h % 2)
                            nc.vector.tensor_copy(
                                outT[p][off:off + 64, nh * 1024:(nh + 1) * 1024],
                                Us[ci][0:64, :])
                            nc.vector.tensor_copy(den_all[:, nh, h],
                                                  Us[ci][64:65, :])
                            del Us[ci]

                    def emit_pos(i):
                        ci, mt = seq[i]
                        h, nh = chunks[ci]
                        if mt == 0:
                            Us[ci] = aps.tile([65, 1024], f32, tag="U", bufs=1,
                                              name=f"U{ci}")
                        p = h // 2
                        off = 64 * (h % 2)
                        ms = slice(mt * 128, (mt + 1) * 128)
                        ps = aps.tile([128, 2, 512], f32, tag="ps", bufs=3)
                        for j in range(2):
                            ns = slice(nh * 1024 + j * 512,
                                       nh * 1024 + (j + 1) * 512)
                            nc.tensor.matmul(ps[:, j], kT[p][off:off + 64, ms],
                                             qT[p][off:off + 64, ns],
                                             start=True, stop=True)
                        ex = epool.tile([128, 1024], bf, tag="ex", bufs=8)
                        psf = ps[:].rearrange("p a b -> p (a b)")
                        if mt in DVE_MT:
                            sc1 = (rs8[:, mt, h:h + 1] if ln_trivial
                                   else SCALE * A_BITS)
                            nc.vector.tensor_scalar(ex[:].bitcast(i16), psf,
                                                    sc1, B_BITS, OP.mult, OP.add)
                        else:
                            esc = rs_k[:, mt, h:h + 1] if ln_trivial else SCALE
                            nc.scalar.activation(ex[:], psf, ACTF.Exp, scale=esc)
                        exs[(ci, mt)] = ex

                    def inj_qtrans(s_, tq):
                        # leftover q transpose via a ps-tag PSUM slot viewed
                        # as bf16 (the tp_ps pool is closed by now)
                        tp = aps.tile([128, 2, 512], f32, tag="ps", bufs=3,
                                      name=f"tp{s_}_{tq}")
                        vw = tp[:, 0, 0:256].bitcast(bf).rearrange(
                            "p (a b) -> p a b", b=128)
                        for i2 in range(4):
                            t = 4 * tq + i2
                            nc.tensor.transpose(
                                vw[:, i2], q_nat[:, t, 2 * s_:2 * s_ + 2, :],
                                ident[:])
                        if tq % 2 == 0:
                            nc.vector.tensor_copy(
                                qT[s_][:, tq * 512:(tq + 1) * 512], vw[:])
                        else:
                            nc.scalar.copy(
                                qT[s_][:, tq * 512:(tq + 1) * 512], vw[:])

                    def inj_norm0(h):
                        # half-0 softmax normalize inside the attention
                        # phase: broadcast matmul into a ps-tag slot + one
                        # DVE multiply; the tail then starts straight with
                        # the projection.
                        rb = aps.tile([128, 2, 512], f32, tag="ps", bufs=3,
                                      name=f"rb0_{h}")
                        for j in range(2):
                            nc.tensor.matmul(rb[0:64, j], ones_t[:],
                                             denr[:, 0, h, j * 512:(j + 1) * 512],
                                             start=True, stop=True)
                        p = h // 2
                        off = 64 * (h % 2)
                        sl = outT[p][off:off + 64, 0:1024]
                        nc.vector.tensor_mul(
                            sl, sl, rb[0:64].rearrange("p a b -> p (a b)"))

                    injections = {66: lambda: emit_recip(0)}
                    for (gg, s_, tq) in ((3, 0, 2), (6, 0, 3), (9, 1, 0),
                                         (12, 1, 1), (15, 1, 2), (18, 1, 3)):
                        injections[gg] = (lambda a, b: lambda: inj_qtrans(a, b))(s_, tq)
                    for k2 in range(HPC):
                        injections[72 + 3 * k2] = (lambda h: lambda: inj_norm0(h))(k2)

                    # batch-3 emission: 6 same-shape scores then 6 same-shape
                    # attnv matmuls; alternating PE geometries every position
                    # costs ~55ns/matmul (array reconfig breaks the ldweights
                    # overlap), so batch to amortize. The 3-position attnv lag
                    # also covers the single-buffered U's evacuation.
                    BATCH = 3
                    prev = []
                    for g in range(0, len(seq), BATCH):
                        cur = list(range(g, min(g + BATCH, len(seq))))
                        for i in cur:
                            emit_pos(i)
                        for i in prev:
                            emit_attnv(*seq[i])
                        fn = injections.get(g)
                        if fn is not None:
                            fn()
                        prev = cur
                    for i in prev:
                        emit_attnv(*seq[i])

                # ---- tail: normalize + proj + out (attention PSUM freed)
                with tc.tile_pool(name="fin_pool", bufs=2) as fpool, \
                     tc.tile_pool(name="nrm_ps", bufs=2, space="PSUM") as nps, \
                     tc.tile_pool(name="prj_ps", bufs=2, space="PSUM") as pps:
                    emit_recip(1)
                    out_r = out.rearrange("(t p) f -> p t f", p=128)
                    fins = {}

                    def emit_norm(nh):
                        # rb = ones^T @ recip row (fp32r broadcast to 64
                        # partitions), then a DVE multiply per head slice
                        cs = slice(nh * 1024, (nh + 1) * 1024)
                        for h in range(HPC):
                            rb = nps.tile([64, 2, 512], f32, tag="rbp",
                                          name=f"rb{nh}_{h}")
                            for j in range(2):
                                nc.tensor.matmul(
                                    rb[:, j],
                                    ones_t[:],
                                    denr[:, nh, h, j * 512:(j + 1) * 512],
                                    start=True, stop=True)
                            p = h // 2
                            off = 64 * (h % 2)
                            sl = outT[p][off:off + 64, cs]
                            nc.vector.tensor_mul(
                                sl, sl, rb[:].rearrange("p a b -> p (a b)"))

                    def emit_proj(t):
                        # proj partial for n-tile t: PSUM -> fp16 SBUF
                        # (DVE/ACT alternating) -> batched DMA every 4 tiles
                        ts_ = slice(t * 128, (t + 1) * 128)
                        po = pps.tile([128, 2, 512], f32, tag="po",
                                      name=f"po{t}")
                        for p in range(2):
                            for j in range(2):
                                nc.tensor.matmul(
                                    po[:, j], outT[p][:, ts_],
                                    wp_s[:, p, j * 512:(j + 1) * 512],
                                    start=(p == 0), stop=(p == 1))
                        g = t // 4
                        if t % 4 == 0:
                            fins[g] = fpool.tile([128, 4, 1024], f16,
                                                 tag="fin", name=f"fin{g}")
                        pof = po[:].rearrange("p a b -> p (a b)")
                        if t % 2 == 0:
                            nc.vector.tensor_copy(fins[g][:, t % 4], pof)
                        else:
                            nc.scalar.copy(fins[g][:, t % 4], pof)
                        if t % 4 == 3:
                            eng = nc.sync if g % 2 == 0 else nc.gpsimd
                            eng.dma_start(out_r[:, g * 4:(g + 1) * 4],
                                          fins.pop(g)[:])

                    for idx in range(8):
                        emit_proj(idx)
                    emit_norm(1)
                    for idx in range(8, 16):
                        emit_proj(idx)

    return nc


def _get_program(ln_trivial: bool):
    key = ln_trivial
    if key not in _nc_cache:
        _install_patch()
        _nc_cache[key] = _build_program(ln_trivial)
    return _nc_cache[key]


def _bf16():
    import ml_dtypes
    return ml_dtypes.bfloat16


def _prep_core_inputs(c, x, qkv_w, q_norm_w, q_norm_b, k_norm_w, k_norm_b,
                      proj_w, ln_trivial):
    b = c // 4
    g = c % 4
    rows = slice(g * DPC, (g + 1) * DPC)
    b16 = _bf16()
    xt = np.ascontiguousarray(x[b].T).astype(b16)           # [C, N]
    wq = np.ascontiguousarray(qkv_w[rows, :].T).astype(b16)  # [C, DPC]
    wk = np.ascontiguousarray(qkv_w[C:2 * C, :][rows, :].T).astype(b16)
    wv = np.ascontiguousarray(qkv_w[2 * C:3 * C, :][rows, :].T).astype(b16)
    wp = np.ascontiguousarray(proj_w[:, rows].T).astype(b16)  # [DPC, C]
    m = {"xt": xt, "wq": wq, "wk": wk, "wv": wv, "wp": wp,
         "ident": np.eye(128, dtype=_bf16()),
         "ones64": np.ones((1, D), dtype=np.float32)}
    if not ln_trivial:
        for nm, arr in (("gqb", q_norm_w), ("bqb", q_norm_b),
                        ("gkb", k_norm_w), ("bkb", k_norm_b)):
            t = np.broadcast_to(arr.astype(np.float32), (128, 4, D))
            m[nm] = np.ascontiguousarray(t)
    return m


def kernel(x, qkv_w, q_norm_w, q_norm_b, k_norm_w, k_norm_b, proj_w, proj_b,
           _trace=False):
    from concourse.bass_utils import run_bass_kernel_spmd

    x = np.asarray(x, dtype=np.float32)
    qkv_w = np.asarray(qkv_w, dtype=np.float32)
    q_norm_w = np.asarray(q_norm_w, dtype=np.float32)
    q_norm_b = np.asarray(q_norm_b, dtype=np.float32)
    k_norm_w = np.asarray(k_norm_w, dtype=np.float32)
    k_norm_b = np.asarray(k_norm_b, dtype=np.float32)
    proj_w = np.asarray(proj_w, dtype=np.float32)
    proj_b = np.asarray(proj_b, dtype=np.float32)

    ln_trivial = (np.all(q_norm_w == 1.0) and np.all(q_norm_b == 0.0)
                  and np.all(k_norm_w == 1.0) and np.all(k_norm_b == 0.0))

    nc = _get_program(ln_trivial)
    in_maps = [
        _prep_core_inputs(c, x, qkv_w, q_norm_w, q_norm_b, k_norm_w,
                          k_norm_b, proj_w, ln_trivial)
        for c in range(NCORES)
    ]
    res = run_bass_kernel_spmd(nc, in_maps, list(range(NCORES)),
                               trace=_trace)
    outs = [res.results[c]["out"] for c in range(NCORES)]
    full = np.empty((B, N, C), dtype=np.float32)
    for b in range(B):
        acc = outs[4 * b].astype(np.float32)
        for g in range(1, 4):
            acc = acc + outs[4 * b + g]
        full[b] = acc + proj_b[None, :]
    if _trace:
        return full, res
    return full
